# revision 1
# baseline (speedup 1.0000x reference)
"""Multi-head causal attention (B=2, S=2048, D=1024, H=16) on 8 trn2 NeuronCores.

Sharding: core c handles batch b = c//4 and head group g = c%4 (heads 4g..4g+3).
Each core computes:
  qkv projection for its 4 heads        [2048,1024] @ [1024,3*256]
  causal attention for its 4 heads      (scoresT layout, softmax w/o max-sub,
                                         causality exploited at 128 blocks)
  partial output projection             ctx_c @ w_out[rows] -> [2048,1024]
Host sums the 4 partial outputs per batch.

Matmuls run in bf16 (f32r available via KERNEL_DT_* env); accumulation is fp32
in PSUM. The softmax denominator comes free from a ones-column appended to v.
"""

import sys
from contextlib import ExitStack

for _p in ("/opt/trn_rl_repo",):
    if _p not in sys.path:
        sys.path.insert(0, _p)

import numpy as np

import concourse.bass as bass  # noqa: F401
import concourse.tile as tile
from concourse import bacc, bass_utils, mybir

B, S, D, H, HD = 2, 2048, 1024, 16, 64
P = 128
NCORES = 8
NT = S // P          # 16 token tiles
KD = D // P          # 8 contraction tiles over D
NB = S // 512        # 4 query blocks of 512
HPC = 4              # heads per core
WCOLS = HPC * HD     # 256 weight columns per core per q/k/v

F32 = mybir.dt.float32
F32R = mybir.dt.float32r
BF16 = mybir.dt.bfloat16
EXP = mybir.ActivationFunctionType.Exp

import os as _os

_DT_NAMES = {"f32r": F32R, "bf16": BF16}
DT_PROJ = _DT_NAMES[_os.environ.get("KERNEL_DT_PROJ", "bf16")]
DT_ATTN = _DT_NAMES[_os.environ.get("KERNEL_DT_ATTN", "bf16")]


def round_f32r(x: np.ndarray) -> np.ndarray:
    """Round fp32 to nearest f32r (11 mantissa bits kept), matching PE HW."""
    b = np.ascontiguousarray(x, dtype=np.float32).view(np.uint32)
    r = (b + np.uint32(0x7FF) + ((b >> np.uint32(12)) & np.uint32(1))) & np.uint32(
        0xFFFFF000
    )
    return r.view(np.float32)


def prep(x: np.ndarray, dt) -> np.ndarray:
    """Convert host fp32 array to the numpy form matching DRAM dtype dt."""
    if dt is F32R:
        return round_f32r(x)
    import ml_dtypes

    return np.ascontiguousarray(x, np.float32).astype(ml_dtypes.bfloat16)


def _emit(tc: tile.TileContext, aps: dict):
    nc = tc.nc
    xT, wq, wk, wv, wo, tri, out = (
        aps["xT"], aps["wq"], aps["wk"], aps["wv"], aps["wo"],
        aps["tri"], aps["out"],
    )

    with ExitStack() as top:
        qk_pool = top.enter_context(tc.tile_pool(name="qk", bufs=4))
        v_pool = top.enter_context(tc.tile_pool(name="v1", bufs=NT))
        ctx_pool = top.enter_context(tc.tile_pool(name="ctxT", bufs=2))
        wo_pool = top.enter_context(tc.tile_pool(name="wo", bufs=2))
        const_pool = top.enter_context(tc.tile_pool(name="const", bufs=1))
        small_pool = top.enter_context(tc.tile_pool(name="small", bufs=2))
        out_pool = top.enter_context(tc.tile_pool(name="outsb", bufs=3))
        exp_pool = top.enter_context(tc.tile_pool(name="expT", bufs=12))
        x_pool = top.enter_context(tc.tile_pool(name="xc", bufs=4 * KD))
        w_pool = top.enter_context(tc.tile_pool(name="w", bufs=3 * KD))
        ps = top.enter_context(tc.tile_pool(name="ps", bufs=4, space="PSUM"))
        ctxps_pool = top.enter_context(
            tc.tile_pool(name="ctxps", bufs=4, space="PSUM")
        )

        # persistent SBUF tiles
        qT = [qk_pool.tile([P, S], DT_ATTN, tag="qk", name=f"qT{i}") for i in range(2)]
        kT = [qk_pool.tile([P, S], DT_ATTN, tag="qk", name=f"kT{i}") for i in range(2)]
        v1 = [
            v_pool.tile([P, HPC * (HD + 1)], DT_ATTN, tag="v1", name=f"v1_{i}")
            for i in range(NT)
        ]
        ctxT = [
            ctx_pool.tile([P, S], DT_PROJ, tag="ctxT", name=f"ctxT{i}")
            for i in range(2)
        ]
        wo_sb = [wo_pool.tile([P, D], DT_PROJ, tag="wo", name=f"wo{i}") for i in range(2)]
        tri_sb = const_pool.tile([P, P], DT_ATTN, tag="tri")
        ones4 = const_pool.tile([P, HPC], F32, tag="ones4")
        nc.vector.memset(ones4[:], 1.0)

        wq_sb = [w_pool.tile([P, WCOLS], DT_PROJ, tag="w", name=f"wq{i}") for i in range(KD)]
        wk_sb = [w_pool.tile([P, WCOLS], DT_PROJ, tag="w", name=f"wk{i}") for i in range(KD)]
        wv_sb = [w_pool.tile([P, WCOLS], DT_PROJ, tag="w", name=f"wv{i}") for i in range(KD)]
        xc = {}

        def dma_xc(kt, nb):
            xc[(kt, nb)] = x_pool.tile(
                [P, 512], DT_PROJ, tag="xc", name=f"xc{kt}_{nb}"
            )
            nc.sync.dma_start(
                xc[(kt, nb)][:], xT[kt * P : (kt + 1) * P, nb * 512 : (nb + 1) * 512]
            )

        # DMA emission: interleave weights with x chunks so compute starts early
        for kt in range(KD):
            nc.sync.dma_start(wq_sb[kt][:], wq[kt * P : (kt + 1) * P, :])
            dma_xc(kt, 0)
        nc.sync.dma_start(tri_sb[:], tri[:])
        for kt in range(KD):
            nc.sync.dma_start(wk_sb[kt][:], wk[kt * P : (kt + 1) * P, :])
            dma_xc(kt, 1)
        for kt in range(KD):
            nc.sync.dma_start(wv_sb[kt][:], wv[kt * P : (kt + 1) * P, :])
            dma_xc(kt, 2)
        for kt in range(KD):
            dma_xc(kt, 3)
        for i in range(2):
            nc.sync.dma_start(wo_sb[i][:], wo[i * P : (i + 1) * P, :])

        # ===== Phase 1: qkv projection (nb-major) ============================
        for nb in range(NB):
            cols = slice(nb * 512, (nb + 1) * 512)
            for p in range(2):
                psq = ps.tile([P, 512], F32, tag="pss")
                for kt in range(KD):
                    nc.tensor.matmul(
                        psq[:],
                        wq_sb[kt][:, p * P : (p + 1) * P],
                        xc[(kt, nb)][:],
                        start=(kt == 0),
                        stop=(kt == KD - 1),
                    )
                nc.scalar.mul(qT[p][:, cols], psq[:], 1.0 / np.sqrt(HD))
            for p in range(2):
                psk = ps.tile([P, 512], F32, tag="pss")
                for kt in range(KD):
                    nc.tensor.matmul(
                        psk[:],
                        wk_sb[kt][:, p * P : (p + 1) * P],
                        xc[(kt, nb)][:],
                        start=(kt == 0),
                        stop=(kt == KD - 1),
                    )
                nc.scalar.copy(kT[p][:, cols], psk[:])
            for tloc in range(4):
                tt = nb * 4 + tloc
                psv = ps.tile([P, 512], F32, tag="pss")
                for kt in range(KD):
                    nc.tensor.matmul(
                        psv[:, 0:WCOLS],
                        xc[(kt, nb)][:, tloc * P : (tloc + 1) * P],
                        wv_sb[kt][:],
                        start=(kt == 0),
                        stop=(kt == KD - 1),
                    )
                v1_view = v1[tt][:].rearrange("p (a c) -> p a c", c=HD + 1)
                nc.scalar.copy(
                    v1_view[:, :, 0:HD],
                    psv[:, 0:WCOLS].rearrange("p (a c) -> p a c", c=HD),
                )
                nc.scalar.copy(
                    v1_view[:, :, HD : HD + 1],
                    ones4[:].rearrange("p (a c) -> p a c", c=1),
                )

        # rec4 tiles pre-allocated and pre-cleared so the memset never sits
        # in the DVE queue at a q-block boundary
        rec4s = [
            small_pool.tile([P, 512], F32, tag="rec4", bufs=NB, name=f"rec4_{i}")
            for i in range(NB)
        ]
        for r in rec4s:
            nc.vector.memset(r[:], 1.0)

        # ===== Phase 2+3: attention (qb-major) with interleaved out-proj =====
        def emit_outproj(qb, last=False):
            # out-proj for the 4 token tiles of q-block qb (emitted one qb
            # late so the next block's QK fills the normalize-chain bubble)
            for tloc in range(4):
                tt = qb * 4 + tloc
                for ob in range(2):
                    pso = ps.tile([P, 512], F32, tag="pss", name=f"pso{tt}_{ob}")
                    for kt2 in range(2):
                        nc.tensor.matmul(
                            pso[:],
                            ctxT[kt2][:, tt * P : (tt + 1) * P],
                            wo_sb[kt2][:, ob * 512 : (ob + 1) * 512],
                            start=(kt2 == 0),
                            stop=(kt2 == 1),
                        )
                    osb = out_pool.tile([P, 512], F32, tag="osb", name=f"osb{tt}_{ob}")
                    if last and ob == 1:
                        nc.vector.tensor_copy(osb[:], pso[:])
                    else:
                        nc.scalar.copy(osb[:], pso[:])
                    nc.sync.dma_start(
                        out[tt * P : (tt + 1) * P, ob * 512 : (ob + 1) * 512], osb[:]
                    )

        for qb in range(NB):
            njt = 4 * qb + 4
            q0 = qb * 512
            rec4 = rec4s[qb]
            ctxps_of = {}
            for h in range(HPC):
                p, off = h // 2, 64 * (h % 2)
                exps = []
                for jt in range(njt):
                    m = jt - 4 * qb
                    lo = P * m if m > 0 else 0
                    pss = ps.tile([P, 512], F32, tag="pss")
                    nc.tensor.matmul(
                        pss[:, lo:512],
                        kT[p][off : off + 64, jt * P : (jt + 1) * P],
                        qT[p][off : off + 64, q0 + lo : q0 + 512],
                        start=True,
                        stop=True,
                    )
                    et = exp_pool.tile([P, 512], DT_ATTN, tag="expT")
                    nc.scalar.activation(et[:, lo:512], pss[:, lo:512], EXP)
                    if m >= 0:  # diagonal 128-block: triangle mask multiply
                        nc.vector.tensor_mul(
                            et[:, lo : lo + P], et[:, lo : lo + P], tri_sb[:]
                        )
                    exps.append((et, lo))

                ctxps = ctxps_pool.tile([65, 512], F32, tag="ctxps")
                ctxps_of[h] = ctxps
                for jt in range(njt):
                    et, lo = exps[jt]
                    nc.tensor.matmul(
                        ctxps[:, lo:512],
                        v1[jt][:, h * 65 : (h + 1) * 65],
                        et[:, lo:512],
                        start=(jt == 0),
                        stop=(jt == njt - 1),
                    )
                nc.vector.tensor_copy(rec4[32 * h : 32 * h + 1, :], ctxps[64:65, :])

            rec4i = small_pool.tile([P, 512], F32, tag="rec4i", name=f"rec4i_{qb}")
            nc.vector.reciprocal(rec4i[:], rec4[:])
            for h in range(HPC):
                p, off = h // 2, 64 * (h % 2)
                rec_s = small_pool.tile([1, 512], F32, tag="rec_s")
                nc.vector.tensor_copy(rec_s[:], rec4i[32 * h : 32 * h + 1, :])
                recb = small_pool.tile([64, 512], F32, tag="recb")
                nc.gpsimd.partition_broadcast(recb[:], rec_s[:], channels=64)
                nc.vector.tensor_mul(
                    ctxT[p][off : off + 64, q0 : q0 + 512],
                    ctxps_of[h][0:64, :],
                    recb[:],
                )

            if qb > 0:
                emit_outproj(qb - 1)
        emit_outproj(NB - 1, last=True)


_BUILD_CACHE = {}


def build():
    if "nc" in _BUILD_CACHE:
        return _BUILD_CACHE["nc"]
    nc = bacc.Bacc("TRN2", target_bir_lowering=False, debug=False)
    aps = {
        "xT": nc.dram_tensor("xT", [D, S], DT_PROJ, kind="ExternalInput").ap(),
        "wq": nc.dram_tensor("wq", [D, WCOLS], DT_PROJ, kind="ExternalInput").ap(),
        "wk": nc.dram_tensor("wk", [D, WCOLS], DT_PROJ, kind="ExternalInput").ap(),
        "wv": nc.dram_tensor("wv", [D, WCOLS], DT_PROJ, kind="ExternalInput").ap(),
        "wo": nc.dram_tensor("wo", [WCOLS, D], DT_PROJ, kind="ExternalInput").ap(),
        "tri": nc.dram_tensor("tri", [P, P], DT_ATTN, kind="ExternalInput").ap(),
        "out": nc.dram_tensor("out", [S, D], F32, kind="ExternalOutput").ap(),
    }
    with tile.TileContext(nc) as tc:
        _emit(tc, aps)
    nc.compile()
    _BUILD_CACHE["nc"] = nc
    return nc


def make_tri() -> np.ndarray:
    """tri[dj, t] = 1 if dj <= t else 0 (causal keep within a 128 block)."""
    dj = np.arange(P)[:, None]
    t = np.arange(P)[None, :]
    return prep(np.where(dj <= t, 1.0, 0.0).astype(np.float32), DT_ATTN)


def make_in_maps(x, w_qkv, w_out):
    tri = make_tri()
    in_maps = []
    for c in range(NCORES):
        b, g = c // 4, c % 4
        cs = slice(g * WCOLS, (g + 1) * WCOLS)
        in_maps.append(
            {
                "xT": prep(x[b].T, DT_PROJ),
                "wq": prep(w_qkv[:, g * WCOLS : (g + 1) * WCOLS], DT_PROJ),
                "wk": prep(w_qkv[:, D + g * WCOLS : D + (g + 1) * WCOLS], DT_PROJ),
                "wv": prep(
                    w_qkv[:, 2 * D + g * WCOLS : 2 * D + (g + 1) * WCOLS], DT_PROJ
                ),
                "wo": prep(w_out[cs, :], DT_PROJ),
                "tri": tri,
            }
        )
    return in_maps


def kernel(x, w_qkv, w_out, _trace=False):
    nc = build()
    in_maps = make_in_maps(
        np.asarray(x, np.float32), np.asarray(w_qkv, np.float32),
        np.asarray(w_out, np.float32),
    )
    res = bass_utils.run_bass_kernel_spmd(
        nc, in_maps, core_ids=list(range(NCORES)), trace=_trace
    )
    outs = [res.results[c]["out"] for c in range(NCORES)]
    full = np.stack(
        [sum(outs[b * 4 : (b + 1) * 4][1:], outs[b * 4]) for b in range(B)], axis=0
    )
    if _trace:
        kernel.last_results = res
    return full.astype(np.float32)



# revision 3
# speedup vs baseline: 1.2365x; 1.2365x over previous
"""Multi-head causal attention (B=2, S=2048, D=1024, H=16) on 8 trn2 NeuronCores.

Sharding: core c handles batch b = c//4 and head group g = c%4 (heads 4g..4g+3).
Each core computes qkv projection, causal attention (scoresT layout) and the
partial output projection for its 4 heads; the host sums the 4 partials per
batch.

v2 layout of work per core, designed around the measured engine costs
(matmul ~N/2.4GHz back-to-back, ACTIVATE ~(N+352)/1.2ns, HAM power throttle
under sustained PE activity):
  - Scalar engine runs ONLY exp, one [128, 1024-lo] ACTIVATE per (head-pair,
    key-tile) straight out of a 2-bank PSUM scores group (both heads of the
    pair side by side) -> 80 big exps instead of 160 small ones + copies.
  - Scores matmuls of a head pair go to PE row groups 0-63/64-127
    (tile_position auto-inferred) and are emitted back-to-back so they run
    CONCURRENTLY on the split PE array (K=64 -> 2x).
  - All PSUM evacuations (qkv, out-proj) are on the Vector engine; softmax
    normalization uses reciprocal_approx_fast + gpsimd partition broadcast.
  - qkv projection of block nb+1 and out-projection of block qb-1 are
    interleaved into attention(qb)'s emission so the Tensor queue always has
    work while exp runs, and exp hides under matmuls globally.
  - 1/sqrt(HD) is folded into wq on the host.
"""

import sys
from contextlib import ExitStack

for _p in ("/opt/trn_rl_repo",):
    if _p not in sys.path:
        sys.path.insert(0, _p)

import numpy as np

import concourse.bass as bass  # noqa: F401
import concourse.tile as tile
from concourse import bacc, bass_utils, mybir

B, S, D, H, HD = 2, 2048, 1024, 16, 64
P = 128
NCORES = 8
NT = S // P          # 16 token tiles
KD = D // P          # 8 contraction tiles over D
NB = S // 512        # 4 query blocks of 512
HPC = 4              # heads per core
WCOLS = HPC * HD     # 256 weight columns per core per q/k/v

F32 = mybir.dt.float32
BF16 = mybir.dt.bfloat16
EXP = mybir.ActivationFunctionType.Exp

DT = BF16


def prep(x: np.ndarray) -> np.ndarray:
    import ml_dtypes

    return np.ascontiguousarray(x, np.float32).astype(ml_dtypes.bfloat16)


def _emit(tc: tile.TileContext, aps: dict):
    nc = tc.nc
    xT, wq, wk, wv, wo, tri, out = (
        aps["xT"], aps["wq"], aps["wk"], aps["wv"], aps["wo"],
        aps["tri"], aps["out"],
    )

    with ExitStack() as top:
        qk_pool = top.enter_context(tc.tile_pool(name="qk", bufs=4))
        v_pool = top.enter_context(tc.tile_pool(name="v1", bufs=NT))
        ctx_pool = top.enter_context(tc.tile_pool(name="ctxT", bufs=2))
        wo_pool = top.enter_context(tc.tile_pool(name="wo", bufs=2))
        const_pool = top.enter_context(tc.tile_pool(name="const", bufs=1))
        small_pool = top.enter_context(tc.tile_pool(name="small", bufs=4))
        out_pool = top.enter_context(tc.tile_pool(name="outsb", bufs=4))
        exp_pool = top.enter_context(tc.tile_pool(name="expT", bufs=5))
        x_pool = top.enter_context(tc.tile_pool(name="xc", bufs=4 * KD))
        w_pool = top.enter_context(tc.tile_pool(name="w", bufs=3 * KD))
        # PSUM: sc 2x[128,1024] (banks 0-3), ctx 2x[65,512] (banks 4-5),
        # pp 2x[128,512] shared by qkv-proj + out-proj fills (banks 6-7)
        sc_pool = top.enter_context(tc.tile_pool(name="sc", bufs=2, space="PSUM"))
        ctxps_pool = top.enter_context(
            tc.tile_pool(name="ctxps", bufs=2, space="PSUM")
        )
        pp_pool = top.enter_context(tc.tile_pool(name="pp", bufs=2, space="PSUM"))

        # persistent SBUF tiles
        qT = [qk_pool.tile([P, S], DT, tag="qk", name=f"qT{i}") for i in range(2)]
        kT = [qk_pool.tile([P, S], DT, tag="qk", name=f"kT{i}") for i in range(2)]
        v1 = [
            v_pool.tile([P, HPC * (HD + 1)], DT, tag="v1", name=f"v1_{i}")
            for i in range(NT)
        ]
        ctxT = [
            ctx_pool.tile([P, S], DT, tag="ctxT", name=f"ctxT{i}")
            for i in range(2)
        ]
        wo_sb = [wo_pool.tile([P, D], DT, tag="wo", name=f"wo{i}") for i in range(2)]
        tri_sb = const_pool.tile([P, P], DT, tag="tri")

        wq_sb = [w_pool.tile([P, WCOLS], DT, tag="w", name=f"wq{i}") for i in range(KD)]
        wk_sb = [w_pool.tile([P, WCOLS], DT, tag="w", name=f"wk{i}") for i in range(KD)]
        wv_sb = [w_pool.tile([P, WCOLS], DT, tag="w", name=f"wv{i}") for i in range(KD)]
        xc = {}

        def dma_xc(kt, nb):
            xc[(kt, nb)] = x_pool.tile(
                [P, 512], DT, tag="xc", name=f"xc{kt}_{nb}"
            )
            nc.sync.dma_start(
                xc[(kt, nb)][:], xT[kt * P : (kt + 1) * P, nb * 512 : (nb + 1) * 512]
            )

        # DMA emission: everything proj(0) needs first, then later x blocks
        for kt in range(KD):
            nc.sync.dma_start(wq_sb[kt][:], wq[kt * P : (kt + 1) * P, :])
            nc.sync.dma_start(wk_sb[kt][:], wk[kt * P : (kt + 1) * P, :])
            nc.sync.dma_start(wv_sb[kt][:], wv[kt * P : (kt + 1) * P, :])
            dma_xc(kt, 0)
        nc.sync.dma_start(tri_sb[:], tri[:])
        for nb in range(1, NB):
            for kt in range(KD):
                dma_xc(kt, nb)
        for i in range(2):
            nc.sync.dma_start(wo_sb[i][:], wo[i * P : (i + 1) * P, :])

        # ones column of v1: memset whole tile once, value region is
        # overwritten by the v-projection evacuations afterwards
        for tt in range(NT):
            nc.vector.memset(v1[tt][:], 1.0)

        # ---- qkv projection fills (one closure per PSUM fill) --------------
        def proj_fills(nb):
            fills = []

            def qk_fill(w_sb, dstT, p, nb=nb):
                def go():
                    ps = pp_pool.tile([P, 512], F32, tag="pp")
                    for kt in range(KD):
                        nc.tensor.matmul(
                            ps[:],
                            w_sb[kt][:, p * P : (p + 1) * P],
                            xc[(kt, nb)][:],
                            start=(kt == 0),
                            stop=(kt == KD - 1),
                        )
                    nc.vector.tensor_copy(
                        dstT[p][:, nb * 512 : (nb + 1) * 512], ps[:]
                    )

                return go

            def v_fill(tloc, nb=nb):
                def go():
                    tt = nb * 4 + tloc
                    ps = pp_pool.tile([P, 512], F32, tag="pp")
                    for kt in range(KD):
                        nc.tensor.matmul(
                            ps[:, 0:WCOLS],
                            xc[(kt, nb)][:, tloc * P : (tloc + 1) * P],
                            wv_sb[kt][:],
                            start=(kt == 0),
                            stop=(kt == KD - 1),
                        )
                    v1_view = v1[tt][:].rearrange("p (a c) -> p a c", c=HD + 1)
                    nc.vector.tensor_copy(
                        v1_view[:, :, 0:HD],
                        ps[:, 0:WCOLS].rearrange("p (a c) -> p a c", c=HD),
                    )

                return go

            for p in range(2):
                fills.append(qk_fill(wq_sb, qT, p))
            for p in range(2):
                fills.append(qk_fill(wk_sb, kT, p))
            for tloc in range(4):
                fills.append(v_fill(tloc))
            return fills

        # ---- out-projection fills for query block qb -----------------------
        def outproj_fills(qb):
            fills = []

            def o_fill(tt, ob):
                def go():
                    pso = pp_pool.tile([P, 512], F32, tag="pp")
                    for kt2 in range(2):
                        nc.tensor.matmul(
                            pso[:],
                            ctxT[kt2][:, tt * P : (tt + 1) * P],
                            wo_sb[kt2][:, ob * 512 : (ob + 1) * 512],
                            start=(kt2 == 0),
                            stop=(kt2 == 1),
                        )
                    osb = out_pool.tile([P, 512], F32, tag="osb")
                    nc.vector.tensor_copy(osb[:], pso[:])
                    nc.sync.dma_start(
                        out[tt * P : (tt + 1) * P, ob * 512 : (ob + 1) * 512], osb[:]
                    )

                return go

            for tloc in range(4):
                for ob in range(2):
                    fills.append(o_fill(qb * 4 + tloc, ob))
            return fills

        # ---- attention for query block qb, with interleaved fills ----------
        def emit_attention(qb, fills):
            q0 = qb * 512
            njt = 4 * qb + 4
            for pair in range(2):
                p = pair
                ctxA = ctxps_pool.tile([65, 512], F32, tag="ctxps")
                ctxB = ctxps_pool.tile([65, 512], F32, tag="ctxps")
                for jt0 in range(0, njt, 2):
                    jts = [jt0] if jt0 + 1 >= njt else [jt0, jt0 + 1]
                    ets = []
                    # scores for the chunk (row-group pairs, 64x128 PE mode)
                    for jt in jts:
                        m = jt - 4 * qb
                        lo = P * m if m > 0 else 0
                        scp = sc_pool.tile([P, 1024], F32, tag="sc")
                        for off in (0, 64):
                            nc.tensor.matmul(
                                scp[:, (off // 64) * 512 + lo : (off // 64) * 512 + 512],
                                kT[p][off : off + 64, jt * P : (jt + 1) * P],
                                qT[p][off : off + 64, q0 + lo : q0 + 512],
                                start=True,
                                stop=True,
                            )
                        et = exp_pool.tile([P, 1024], DT, tag="expT")
                        nc.scalar.activation(et[:, lo:1024], scp[:, lo:1024], EXP)
                        if m >= 0:  # diagonal block: triangle mask multiply
                            for half in range(2):
                                h0 = half * 512
                                nc.vector.tensor_mul(
                                    et[:, h0 + lo : h0 + lo + P],
                                    et[:, h0 + lo : h0 + lo + P],
                                    tri_sb[:],
                                )
                        ets.append((et, lo, jt))
                    # interleaved fill work keeps the PE busy during exp
                    if fills:
                        fills.pop(0)()
                    # attn @ v for the chunk (128x128 PE mode)
                    for et, lo, jt in ets:
                        for half, ctx in ((0, ctxA), (1, ctxB)):
                            h = 2 * pair + half
                            nc.tensor.matmul(
                                ctx[:, lo:512],
                                v1[jt][:, h * 65 : (h + 1) * 65],
                                et[:, half * 512 + lo : half * 512 + 512],
                                start=(jt == 0),
                                stop=(jt == njt - 1),
                                skip_group_check=True,
                            )
                # normalize: rec = 1/denominator, broadcast, scale into ctxT
                rec = small_pool.tile([1, 1024], F32, tag="rec")
                nc.vector.tensor_copy(rec[:, 0:512], ctxA[64:65, :])
                nc.vector.tensor_copy(rec[:, 512:1024], ctxB[64:65, :])
                reci = small_pool.tile([1, 1024], F32, tag="reci")
                nc.vector.reciprocal_approx_fast(reci[:], rec[:])
                recb = small_pool.tile([64, 1024], F32, tag="recb")
                nc.gpsimd.partition_broadcast(recb[:], reci[:], channels=64)
                for half, ctx in ((0, ctxA), (1, ctxB)):
                    nc.vector.tensor_mul(
                        ctxT[p][64 * half : 64 * half + 64, q0 : q0 + 512],
                        ctx[0:64, :],
                        recb[:, half * 512 : half * 512 + 512],
                    )

        # ===== schedule ======================================================
        for f in proj_fills(0):
            f()
        for qb in range(NB):
            fills = []
            if qb + 1 < NB:
                fills += proj_fills(qb + 1)
            if qb > 0:
                fills += outproj_fills(qb - 1)
            emit_attention(qb, fills)
            for f in fills:  # anything not consumed by the chunk slots
                f()
        for f in outproj_fills(NB - 1):
            f()


_BUILD_CACHE = {}


def build():
    if "nc" in _BUILD_CACHE:
        return _BUILD_CACHE["nc"]
    nc = bacc.Bacc("TRN2", target_bir_lowering=False, debug=False)
    aps = {
        "xT": nc.dram_tensor("xT", [D, S], DT, kind="ExternalInput").ap(),
        "wq": nc.dram_tensor("wq", [D, WCOLS], DT, kind="ExternalInput").ap(),
        "wk": nc.dram_tensor("wk", [D, WCOLS], DT, kind="ExternalInput").ap(),
        "wv": nc.dram_tensor("wv", [D, WCOLS], DT, kind="ExternalInput").ap(),
        "wo": nc.dram_tensor("wo", [WCOLS, D], DT, kind="ExternalInput").ap(),
        "tri": nc.dram_tensor("tri", [P, P], DT, kind="ExternalInput").ap(),
        "out": nc.dram_tensor("out", [S, D], F32, kind="ExternalOutput").ap(),
    }
    with tile.TileContext(nc) as tc:
        _emit(tc, aps)
    nc.compile()
    _BUILD_CACHE["nc"] = nc
    return nc


def make_tri() -> np.ndarray:
    """tri[dj, t] = 1 if dj <= t else 0 (causal keep within a 128 block)."""
    dj = np.arange(P)[:, None]
    t = np.arange(P)[None, :]
    return prep(np.where(dj <= t, 1.0, 0.0).astype(np.float32))


def make_in_maps(x, w_qkv, w_out):
    tri = make_tri()
    scale = 1.0 / np.sqrt(HD)
    in_maps = []
    for c in range(NCORES):
        b, g = c // 4, c % 4
        cs = slice(g * WCOLS, (g + 1) * WCOLS)
        in_maps.append(
            {
                "xT": prep(x[b].T),
                "wq": prep(w_qkv[:, g * WCOLS : (g + 1) * WCOLS] * scale),
                "wk": prep(w_qkv[:, D + g * WCOLS : D + (g + 1) * WCOLS]),
                "wv": prep(w_qkv[:, 2 * D + g * WCOLS : 2 * D + (g + 1) * WCOLS]),
                "wo": prep(w_out[cs, :]),
                "tri": tri,
            }
        )
    return in_maps


def kernel(x, w_qkv, w_out, _trace=False):
    nc = build()
    in_maps = make_in_maps(
        np.asarray(x, np.float32), np.asarray(w_qkv, np.float32),
        np.asarray(w_out, np.float32),
    )
    res = bass_utils.run_bass_kernel_spmd(
        nc, in_maps, core_ids=list(range(NCORES)), trace=_trace
    )
    outs = [res.results[c]["out"] for c in range(NCORES)]
    full = np.stack(
        [sum(outs[b * 4 : (b + 1) * 4][1:], outs[b * 4]) for b in range(B)], axis=0
    )
    if _trace:
        kernel.last_results = res
    return full.astype(np.float32)
